# revision 12
# baseline (speedup 1.0000x reference)
"""Trainium2 Bass kernel for nn_DecoderBlock (B=2, L=2048, E=1024, H=16, D=64, DFF=4096).

Strategy (8 NeuronCores, SPMD):
  - Attention is head-parallel: core c computes heads {2c, 2c+1} for all 4096
    tokens (uniform program across cores), including the out-proj partial
    contribution of its 128 hd-rows of Wo -- exchanged with a single AllToAll
    so each core ends up with the full attention output for its own 512-token
    slice.
  - Everything else (norms, residuals, GLU FFN) is token-parallel on the
    core's own 512 tokens.
  - Residual stream is kept transposed ([E on partitions, tokens free]) so
    all matmuls use natural weight layouts and biases are per-partition.
  - ALiBi is folded into the QK^T matmul as a rank-2 extension of the
    contraction dim (rows 64/65 of Qhat/Khat); causal masking multiplies
    the two diagonal-straddling k-tiles by constant 0/1 masks after exp;
    fully-masked tiles are skipped. Softmax needs no max-subtraction
    (exponents bounded by ~|S|/8 <= ~3) and normalization divides the
    V-weighted sums per head.
  - All matmuls run in float32r (full PE speed at >=256 moving free dim).
"""

import sys
import types

import numpy as np

sys.path.insert(0, "/opt/trn_rl_repo")
sys.path.insert(0, "/opt/pypackages")

import concourse.bass as bass
import concourse.mybir as mybir
from concourse import bacc
import concourse.tile as tile
from concourse.bass_utils import run_bass_kernel_spmd

F32 = mybir.dt.float32
F32R = mybir.dt.float32r
AF = mybir.ActivationFunctionType
ALU = mybir.AluOpType

B, L, E, H, D, DFF = 2, 2048, 1024, 16, 64, 4096
T = B * L            # 4096 flat tokens
NC_ = 8              # cores
TOK = T // NC_       # 512 own tokens per core
ET = E // 128        # 8 e-tiles
DT = DFF // 128      # 32 dff-tiles
QC = 256             # q-chunk width in attention
NQC = L // QC        # 8 q-chunks per sequence

_CACHE = {}


def install_ntff_hook():
    """Synthesize antenv.axon_hooks so trace=True can profile via libaxon_pjrt."""
    try:
        from antenv.axon_hooks import get_axon_ntff_profile_hook  # noqa
        return
    except ImportError:
        pass
    try:
        import antenv
        mod = types.ModuleType("antenv.axon_hooks")
        mod._hook = None
        mod.set_axon_ntff_profile_hook = lambda h: setattr(mod, "_hook", h)
        mod.get_axon_ntff_profile_hook = lambda: mod._hook
        sys.modules["antenv.axon_hooks"] = mod
        antenv.axon_hooks = mod
        if "/root/.axon_site" not in sys.path:
            sys.path.insert(0, "/root/.axon_site")
        from trn_agent_boot.trn_boot import _ntff_profile_via_ctypes
        mod.set_axon_ntff_profile_hook(
            _ntff_profile_via_ctypes("/opt/axon/libaxon_pjrt.so")
        )
    except Exception:
        pass


def build_nc():
    nc = bacc.Bacc("TRN2", target_bir_lowering=False, debug=False)

    # ---- I/O ----
    xt_full = nc.dram_tensor("xt_full", [E, T], F32R, kind="ExternalInput")
    xt_own = nc.dram_tensor("xt_own", [E, TOK], F32R, kind="ExternalInput")
    wq = nc.dram_tensor("wq", [E, 128], F32R, kind="ExternalInput")
    wk = nc.dram_tensor("wk", [E, 128], F32R, kind="ExternalInput")
    wv = nc.dram_tensor("wv", [E, 128], F32R, kind="ExternalInput")
    bqkv = nc.dram_tensor("bqkv", [128, 3], F32, kind="ExternalInput")
    wo = nc.dram_tensor("wo", [E, E], F32R, kind="ExternalInput")
    ww = nc.dram_tensor("ww", [E, DFF], F32R, kind="ExternalInput")
    wvf = nc.dram_tensor("wvf", [E, DFF], F32R, kind="ExternalInput")
    wout = nc.dram_tensor("wout", [DFF, E], F32R, kind="ExternalInput")
    posd = nc.dram_tensor("posd", [1, T], F32R, kind="ExternalInput")
    slopesd = nc.dram_tensor("slopesd", [2, 1], F32, kind="ExternalInput")
    onesd = nc.dram_tensor("onesd", [128, 1], F32R, kind="ExternalInput")
    extkd = nc.dram_tensor("extkd", [2, T], F32R, kind="ExternalInput")
    extqd = nc.dram_tensor("extqd", [2, T], F32R, kind="ExternalInput")
    extsckd = nc.dram_tensor("extsckd", [2, 2], F32, kind="ExternalInput")
    extscqd = nc.dram_tensor("extscqd", [2, 2], F32, kind="ExternalInput")
    mask0d = nc.dram_tensor("mask0d", [128, QC], F32R, kind="ExternalInput")
    mask1d = nc.dram_tensor("mask1d", [128, QC], F32R, kind="ExternalInput")
    identd = nc.dram_tensor("identd", [128, 128], F32R, kind="ExternalInput")

    y = nc.dram_tensor("y", [TOK, E], F32, kind="ExternalOutput")

    a2a_in = nc.dram_tensor("a2a_in", [NC_, TOK, 128], F32R)
    a2a_out = nc.dram_tensor("a2a_out", [NC_, TOK, 128], F32R)

    with tile.TileContext(nc) as tc:
        # ---------- constants (whole kernel) ----------
        from contextlib import ExitStack
        estack = ExitStack()
        const = estack.enter_context(tc.tile_pool(name="const", bufs=1))
        ones_sb = const.tile([128, 1], F32R)
        nc.sync.dma_start(out=ones_sb, in_=onesd[:])
        mask_sb = [const.tile([128, QC], F32R, name=f"mask{i}", tag=f"mask{i}") for i in range(2)]
        nc.sync.dma_start(out=mask_sb[0], in_=mask0d[:])
        nc.sync.dma_start(out=mask_sb[1], in_=mask1d[:])
        ident_sb = const.tile([128, 128], F32R)
        nc.sync.dma_start(out=ident_sb, in_=identd[:])
        bqkv_sb = const.tile([128, 3], F32)
        nc.sync.dma_start(out=bqkv_sb, in_=bqkv[:])

        # ---------- persistent attention state ----------
        att_stack = ExitStack()
        att_pool = att_stack.enter_context(tc.tile_pool(name="att", bufs=1))
        qhat = [att_pool.tile([66, T], F32R, name=f"qhat{h}", tag=f"qhat{h}") for h in range(2)]
        khat = [att_pool.tile([66, T], F32R, name=f"khat{h}", tag=f"khat{h}") for h in range(2)]
        vnat = att_pool.tile([128, T], F32R, tag="vnat")  # cols jt*128+d2

        # alibi extension rows (written as [2, T] blocks at partition base 64)
        with tc.tile_pool(name="extp", bufs=1) as extp:
            extk_sb = extp.tile([2, T], F32R)
            nc.sync.dma_start(out=extk_sb, in_=extkd[:])
            extq_sb = extp.tile([2, T], F32R)
            nc.sync.dma_start(out=extq_sb, in_=extqd[:])
            extsck_sb = extp.tile([2, 2], F32)
            nc.sync.dma_start(out=extsck_sb, in_=extsckd[:])
            extscq_sb = extp.tile([2, 2], F32)
            nc.sync.dma_start(out=extscq_sb, in_=extscqd[:])
            for hh in range(2):
                nc.vector.tensor_scalar(
                    out=qhat[hh][64:66, :], in0=extq_sb[:],
                    scalar1=extscq_sb[:, hh:hh + 1], scalar2=None, op0=ALU.mult,
                )
                nc.vector.tensor_scalar(
                    out=khat[hh][64:66, :], in0=extk_sb[:],
                    scalar1=extsck_sb[:, hh:hh + 1], scalar2=None, op0=ALU.mult,
                )

        # ================= Phase 1: rmsnorm1 + QKV projections =================
        with (
            tc.tile_pool(name="p1", bufs=2) as p1,
            tc.tile_pool(name="p1s", bufs=1) as p1s,
            tc.tile_pool(name="p1ps", bufs=2, space="PSUM") as p1ps,
        ):
            wq_sb = p1s.tile([128, ET, 128], F32R, tag="wq")
            wk_sb = p1s.tile([128, ET, 128], F32R, tag="wk")
            wv_sb = p1s.tile([128, ET, 128], F32R, tag="wv")
            for e in range(ET):
                nc.sync.dma_start(out=wq_sb[:, e, :], in_=wq[128 * e:128 * e + 128, :])
                nc.sync.dma_start(out=wk_sb[:, e, :], in_=wk[128 * e:128 * e + 128, :])
                nc.sync.dma_start(out=wv_sb[:, e, :], in_=wv[128 * e:128 * e + 128, :])

            for tci in range(T // 512):
                c0 = 512 * tci
                xtc = p1.tile([128, ET, 512], F32R, tag="xtc")
                for e in range(ET):
                    nc.sync.dma_start(
                        out=xtc[:, e, :], in_=xt_full[128 * e:128 * e + 128, c0:c0 + 512]
                    )
                ss_ps = p1ps.tile([1, 512], F32, tag="ss")
                for e in range(ET):
                    sq_e = p1.tile([128, 512], F32R, tag="sq", name="sq_e")
                    nc.scalar.activation(sq_e[:], xtc[:, e, :], AF.Square)
                    nc.tensor.matmul(
                        ss_ps[:], ones_sb[:], sq_e[:],
                        start=(e == 0), stop=(e == ET - 1),
                    )
                rinv = p1.tile([1, 512], F32R, tag="rinv")
                with nc.allow_low_precision(reason="f32r same bits as f32"):
                    nc.vector.reciprocal(rinv[:], ss_ps[:])
                r = p1.tile([1, 512], F32R, tag="r")
                nc.scalar.activation(r[:], rinv[:], AF.Sqrt, scale=float(E))
                rb = p1.tile([128, 512], F32R, tag="rb")
                nc.gpsimd.partition_broadcast(rb[:], r[:])
                h_t = p1.tile([128, ET, 512], F32R, tag="h")
                for e in range(ET):
                    nc.vector.tensor_mul(h_t[:, e, :], xtc[:, e, :], rb[:])

                # Q
                q_ps = p1ps.tile([128, 512], F32, tag="qkv")
                for e in range(ET):
                    nc.tensor.matmul(
                        q_ps[:], wq_sb[:, e, :], h_t[:, e, :],
                        start=(e == 0), stop=(e == ET - 1),
                    )
                for hh in range(2):
                    nc.scalar.activation(
                        qhat[hh][0:64, c0:c0 + 512], q_ps[64 * hh:64 * hh + 64, :],
                        AF.Identity, bias=bqkv_sb[64 * hh:64 * hh + 64, 0:1], scale=0.125,
                    )
                # K
                k_ps = p1ps.tile([128, 512], F32, tag="qkv")
                for e in range(ET):
                    nc.tensor.matmul(
                        k_ps[:], wk_sb[:, e, :], h_t[:, e, :],
                        start=(e == 0), stop=(e == ET - 1),
                    )
                for hh in range(2):
                    nc.scalar.activation(
                        khat[hh][0:64, c0:c0 + 512], k_ps[64 * hh:64 * hh + 64, :],
                        AF.Identity, bias=bqkv_sb[64 * hh:64 * hh + 64, 1:2],
                    )
                # V -> natural layout via transpose
                v_ps = p1ps.tile([128, 512], F32, tag="qkv")
                for e in range(ET):
                    nc.tensor.matmul(
                        v_ps[:], wv_sb[:, e, :], h_t[:, e, :],
                        start=(e == 0), stop=(e == ET - 1),
                    )
                vt_sb = p1.tile([128, 512], F32R, tag="vt")
                nc.scalar.activation(vt_sb[:], v_ps[:], AF.Identity, bias=bqkv_sb[:, 2:3])
                for j in range(4):
                    vtr_ps = p1ps.tile([128, 128], F32R, tag="vtr")
                    nc.tensor.transpose(
                        vtr_ps[:], vt_sb[:, 128 * j:128 * j + 128], ident_sb[:]
                    )
                    jt = 4 * tci + j
                    nc.vector.tensor_copy(vnat[:, 128 * jt:128 * jt + 128], vtr_ps[:])

        # ================= Phase 2: attention =================
        with (
            tc.tile_pool(name="p2", bufs=4) as p2,
            tc.tile_pool(name="p2b", bufs=2) as p2b,
            tc.tile_pool(name="p2ps", bufs=2, space="PSUM") as p2ps,
            tc.tile_pool(name="p2ps1", bufs=1, space="PSUM") as p2ps1,
        ):
            for hh in range(2):
                for s in range(B):
                    for qc in range(NQC):
                        q0 = s * L + QC * qc
                        nkt = 2 * (qc + 1)
                        ctx_ps = p2ps.tile([64, QC], F32, tag="ctx")
                        rs_ps = p2ps.tile([1, QC], F32, tag="rs")
                        for kt in range(nkt):
                            koff = s * L + 128 * kt
                            s_ps = p2ps.tile([128, QC], F32, tag="sps")
                            nc.tensor.matmul(
                                s_ps[:], khat[hh][:, koff:koff + 128],
                                qhat[hh][:, q0:q0 + QC], start=True, stop=True,
                            )
                            a_sb = p2.tile([128, QC], F32R, tag="a")
                            diag = kt >= 2 * qc
                            if diag:
                                # clamp so exp of to-be-masked entries can't overflow
                                s_cl = p2.tile([128, QC], F32R, tag="scl", name="s_cl")
                                nc.vector.tensor_scalar(
                                    out=s_cl[:], in0=s_ps[:], scalar1=80.0,
                                    scalar2=None, op0=ALU.min,
                                )
                                nc.scalar.activation(a_sb[:], s_cl[:], AF.Exp)
                                nc.vector.tensor_mul(
                                    a_sb[:], a_sb[:], mask_sb[kt - 2 * qc][:]
                                )
                            else:
                                nc.scalar.activation(a_sb[:], s_ps[:], AF.Exp)
                            nc.tensor.matmul(
                                rs_ps[:], ones_sb[:], a_sb[:],
                                start=(kt == 0), stop=(kt == nkt - 1),
                            )
                            jt = s * (L // 128) + kt
                            nc.tensor.matmul(
                                ctx_ps[:],
                                vnat[:, 128 * jt + 64 * hh:128 * jt + 64 * hh + 64],
                                a_sb[:], start=(kt == 0), stop=(kt == nkt - 1),
                            )
                        rinv_a = p2b.tile([1, QC], F32R, tag="rinva")
                        with nc.allow_low_precision(reason="f32r same bits"):
                            nc.vector.reciprocal(rinv_a[:], rs_ps[:])
                        bca = p2b.tile([64, QC], F32R, tag="bca")
                        nc.gpsimd.partition_broadcast(bca[:], rinv_a[:])
                        ctxn = p2b.tile([64, QC], F32R, tag="ctxn")
                        nc.vector.tensor_mul(ctxn[:], ctx_ps[:], bca[:])
                        for half in range(2):
                            tr_ps = p2ps1.tile([128, 64], F32R, tag="tr")
                            nc.tensor.transpose(
                                tr_ps[:], ctxn[:, 128 * half:128 * half + 128],
                                ident_sb[0:64, 0:64],
                            )
                            ctx_nat = p2b.tile([128, 64], F32R, tag="cnat")
                            nc.vector.tensor_copy(ctx_nat[:], tr_ps[:])
                            tglob = s * L + QC * qc + 128 * half
                            j = tglob // TOK
                            tloc = tglob % TOK
                            nc.sync.dma_start(
                                out=a2a_in[j, tloc:tloc + 128, 64 * hh:64 * hh + 64],
                                in_=ctx_nat[:],
                            )

        # ================= AllToAll =================
        nc.gpsimd.collective_compute(
            "AllToAll", ALU.bypass,
            replica_groups=[list(range(NC_))],
            ins=[a2a_in[:]], outs=[a2a_out[:]],
        )

        att_stack.close()

        # ================= Phase 3a: out-proj + residual + cross stage ========
        x_stack = ExitStack()
        x_pool = x_stack.enter_context(tc.tile_pool(name="xp", bufs=1))
        x3 = x_pool.tile([128, ET, TOK], F32R, tag="x3")
        h3 = x_pool.tile([128, ET, TOK], F32R, tag="h3")

        with (
            tc.tile_pool(name="p3", bufs=3) as p3,
            tc.tile_pool(name="p3s", bufs=1) as p3s,
            tc.tile_pool(name="p3w", bufs=4) as p3w,
            tc.tile_pool(name="p3ps", bufs=2, space="PSUM") as p3ps,
        ):
            # received ctx -> transposed [hd, tok]
            cxt = p3s.tile([128, ET, TOK], F32R, tag="cxt")
            for p in range(NC_):
                for tt in range(TOK // 128):
                    rb_sb = p3.tile([128, 128], F32R, tag="rbsb")
                    nc.sync.dma_start(
                        out=rb_sb[:], in_=a2a_out[p, 128 * tt:128 * tt + 128, :]
                    )
                    tr_ps = p3ps.tile([128, 128], F32R, tag="tr")
                    nc.tensor.transpose(tr_ps[:], rb_sb[:], ident_sb[:])
                    nc.vector.tensor_copy(
                        cxt[:, p, 128 * tt:128 * tt + 128], tr_ps[:]
                    )
            # own residual stream
            xo_sb = p3s.tile([128, ET, TOK], F32R, tag="xo")
            for e in range(ET):
                nc.sync.dma_start(
                    out=xo_sb[:, e, :], in_=xt_own[128 * e:128 * e + 128, :]
                )
            # out-proj + residual -> x2 (stored into x3 buffer temporarily? keep x2)
            x2 = p3s.tile([128, ET, TOK], F32R, tag="x2")
            for e in range(ET):
                op_ps = p3ps.tile([128, TOK], F32, tag="op")
                for p in range(ET):
                    wo_t = p3w.tile([128, 128], F32R, tag="wot")
                    nc.sync.dma_start(
                        out=wo_t[:],
                        in_=wo[128 * p:128 * p + 128, 128 * e:128 * e + 128],
                    )
                    nc.tensor.matmul(
                        op_ps[:], wo_t[:], cxt[:, p, :],
                        start=(p == 0), stop=(p == ET - 1),
                    )
                nc.vector.tensor_add(x2[:, e, :], op_ps[:], xo_sb[:, e, :])

            # cross-attn stage: x3 = x2 * (1 + rms_scale)
            ss2 = p3ps.tile([1, TOK], F32, tag="ss2")
            for e in range(ET):
                sq2 = p3.tile([128, TOK], F32R, tag="sq2")
                nc.scalar.activation(sq2[:], x2[:, e, :], AF.Square)
                nc.tensor.matmul(
                    ss2[:], ones_sb[:], sq2[:], start=(e == 0), stop=(e == ET - 1)
                )
            rinv2 = p3.tile([1, TOK], F32R, tag="rinv2")
            with nc.allow_low_precision(reason="f32r same bits"):
                nc.vector.reciprocal(rinv2[:], ss2[:])
            r2 = p3.tile([1, TOK], F32R, tag="r2")
            nc.scalar.activation(r2[:], rinv2[:], AF.Sqrt, scale=float(E))
            r2p = p3.tile([1, TOK], F32R, tag="r2p")
            nc.vector.tensor_scalar(
                out=r2p[:], in0=r2[:], scalar1=1.0, scalar2=None, op0=ALU.add
            )
            rb2 = p3.tile([128, TOK], F32R, tag="rb2")
            nc.gpsimd.partition_broadcast(rb2[:], r2p[:])
            for e in range(ET):
                nc.vector.tensor_mul(x3[:, e, :], x2[:, e, :], rb2[:])

            # rmsnorm3 -> h3
            ss3 = p3ps.tile([1, TOK], F32, tag="ss2")
            for e in range(ET):
                sq3 = p3.tile([128, TOK], F32R, tag="sq2")
                nc.scalar.activation(sq3[:], x3[:, e, :], AF.Square)
                nc.tensor.matmul(
                    ss3[:], ones_sb[:], sq3[:], start=(e == 0), stop=(e == ET - 1)
                )
            rinv3 = p3.tile([1, TOK], F32R, tag="rinv2")
            with nc.allow_low_precision(reason="f32r same bits"):
                nc.vector.reciprocal(rinv3[:], ss3[:])
            r3 = p3.tile([1, TOK], F32R, tag="r2")
            nc.scalar.activation(r3[:], rinv3[:], AF.Sqrt, scale=float(E))
            rb3 = p3.tile([128, TOK], F32R, tag="rb2")
            nc.gpsimd.partition_broadcast(rb3[:], r3[:])
            for e in range(ET):
                nc.vector.tensor_mul(h3[:, e, :], x3[:, e, :], rb3[:])

        # ================= Phase 3b: GLU FFN =================
        with (
            tc.tile_pool(name="g", bufs=1) as gpool,
            tc.tile_pool(name="f3", bufs=2) as f3,
            tc.tile_pool(name="f3w", bufs=2) as f3w,
            tc.tile_pool(name="f3ps", bufs=3, space="PSUM") as f3ps,
        ):
            g = gpool.tile([128, DT, TOK], F32R, tag="g")
            NBLK = 2  # n-tiles per weight chunk
            for nb in range(DT // NBLK):
                ww_c = f3w.tile([128, ET, NBLK * 128], F32R, tag="wwc")
                wv_c = f3w.tile([128, ET, NBLK * 128], F32R, tag="wvc")
                for e in range(ET):
                    nc.sync.dma_start(
                        out=ww_c[:, e, :],
                        in_=ww[128 * e:128 * e + 128,
                               NBLK * 128 * nb:NBLK * 128 * (nb + 1)],
                    )
                    nc.sync.dma_start(
                        out=wv_c[:, e, :],
                        in_=wvf[128 * e:128 * e + 128,
                                NBLK * 128 * nb:NBLK * 128 * (nb + 1)],
                    )
                for j in range(NBLK):
                    n = NBLK * nb + j
                    u_ps = f3ps.tile([128, TOK], F32, tag="mm")
                    for e in range(ET):
                        nc.tensor.matmul(
                            u_ps[:], ww_c[:, e, 128 * j:128 * j + 128], h3[:, e, :],
                            start=(e == 0), stop=(e == ET - 1),
                        )
                    nc.scalar.activation(g[:, n, :], u_ps[:], AF.Gelu)
                    v_ps = f3ps.tile([128, TOK], F32, tag="mm")
                    for e in range(ET):
                        nc.tensor.matmul(
                            v_ps[:], wv_c[:, e, 128 * j:128 * j + 128], h3[:, e, :],
                            start=(e == 0), stop=(e == ET - 1),
                        )
                    vtmp = f3.tile([128, TOK], F32R, tag="vtmp")
                    nc.vector.tensor_copy(vtmp[:], v_ps[:])
                    nc.vector.tensor_mul(g[:, n, :], g[:, n, :], vtmp[:])

            # Wout + residual -> x4 (reuse h3 buffer? keep separate small)
            x4 = gpool.tile([128, ET, TOK], F32R, tag="x4")
            for e in range(ET):
                f_ps = f3ps.tile([128, TOK], F32, tag="mm")
                for d in range(DT):
                    wt = f3w.tile([128, 128], F32R, tag="wot2")
                    nc.sync.dma_start(
                        out=wt[:],
                        in_=wout[128 * d:128 * d + 128, 128 * e:128 * e + 128],
                    )
                    nc.tensor.matmul(
                        f_ps[:], wt[:], g[:, d, :],
                        start=(d == 0), stop=(d == DT - 1),
                    )
                nc.vector.tensor_add(x4[:, e, :], f_ps[:], x3[:, e, :])

            # output transpose + DMA
            for tt in range(TOK // 128):
                out_nat = f3.tile([128, E], F32, tag="onat")
                for e in range(ET):
                    tr_ps = f3ps.tile([128, 128], F32R, tag="otr")
                    nc.tensor.transpose(
                        tr_ps[:], x4[:, e, 128 * tt:128 * tt + 128], ident_sb[:]
                    )
                    nc.vector.tensor_copy(out_nat[:, 128 * e:128 * e + 128], tr_ps[:])
                nc.sync.dma_start(out=y[128 * tt:128 * tt + 128, :], in_=out_nat[:])

        x_stack.close()
        estack.close()

    nc.finalize()
    return nc


def make_in_maps(X, Wqkv, bqkv, Wo_sa, Ww, Wv, Wout):
    Xf = np.ascontiguousarray(X.reshape(T, E).astype(np.float32))
    XT = np.ascontiguousarray(Xf.T)  # [E, T]
    Wr = Wqkv.reshape(E, H, 3, D)
    br = bqkv.reshape(H, 3, D)
    pos = (np.arange(T, dtype=np.float32) % L)[None, :]
    slopes_all = (2.0 ** (-np.linspace(1.0, 8.0, H))).astype(np.float32)
    ones_col = np.ones([128, 1], np.float32)
    onesrow = np.ones([1, T], np.float32)
    extk = np.concatenate([pos, onesrow], axis=0).astype(np.float32)   # [2, T]
    extq = np.concatenate([onesrow, pos], axis=0).astype(np.float32)
    p_i = np.arange(128)[:, None]
    f_i = np.arange(QC)[None, :]
    mask0 = (f_i - p_i >= 0).astype(np.float32)
    mask1 = (f_i - p_i - 128 >= 0).astype(np.float32)
    ident = np.eye(128, dtype=np.float32)

    in_maps = []
    for c in range(NC_):
        h0 = 2 * c
        wq_p = np.ascontiguousarray(
            np.concatenate([Wr[:, h0, 0, :], Wr[:, h0 + 1, 0, :]], axis=1)
        )
        wk_p = np.ascontiguousarray(
            np.concatenate([Wr[:, h0, 1, :], Wr[:, h0 + 1, 1, :]], axis=1)
        )
        wv_p = np.ascontiguousarray(
            np.concatenate([Wr[:, h0, 2, :], Wr[:, h0 + 1, 2, :]], axis=1)
        )
        bq_p = np.concatenate([br[h0, 0], br[h0 + 1, 0]]) * 0.125
        bk_p = np.concatenate([br[h0, 1], br[h0 + 1, 1]])
        bv_p = np.concatenate([br[h0, 2], br[h0 + 1, 2]])
        bq3 = np.stack([bq_p, bk_p, bv_p], axis=1).astype(np.float32)  # [128,3]
        xt_own = np.ascontiguousarray(XT[:, TOK * c:TOK * (c + 1)])
        in_maps.append({
            "xt_full": XT,
            "xt_own": xt_own,
            "wq": wq_p.astype(np.float32),
            "wk": wk_p.astype(np.float32),
            "wv": wv_p.astype(np.float32),
            "bqkv": bq3,
            "wo": np.ascontiguousarray(Wo_sa.astype(np.float32)),
            "ww": np.ascontiguousarray(Ww.astype(np.float32)),
            "wvf": np.ascontiguousarray(Wv.astype(np.float32)),
            "wout": np.ascontiguousarray(Wout.astype(np.float32)),
            "posd": pos,
            "slopesd": slopes_all[h0:h0 + 2].reshape(2, 1),
            "onesd": ones_col,
            "extkd": extk,
            "extqd": extq,
            "extsckd": np.stack(
                [[slopes_all[h0], 1.0], [slopes_all[h0 + 1], 1.0]], axis=1
            ).astype(np.float32),
            "extscqd": np.stack(
                [[1.0, -slopes_all[h0]], [1.0, -slopes_all[h0 + 1]]], axis=1
            ).astype(np.float32),
            "mask0d": mask0,
            "mask1d": mask1,
            "identd": ident,
        })
    return in_maps


def kernel(**inputs) -> np.ndarray:
    out, _ = run(inputs, trace=False)
    return out


def run(inputs, trace=False):
    """bo_sa is folded into xt_own host-side (it is a constant per-channel
    add to the residual stream right where xt_own enters)."""
    if "nc" not in _CACHE:
        _CACHE["nc"] = build_nc()
    nc = _CACHE["nc"]
    in_maps = make_in_maps(
        inputs["X"], inputs["Wqkv"], inputs["bqkv"], inputs["Wo_sa"],
        inputs["Ww"], inputs["Wv"], inputs["Wout"],
    )
    bo = inputs["bo_sa"].astype(np.float32)  # [E]
    for m in in_maps:
        m["xt_own"] = np.ascontiguousarray(m["xt_own"] + bo[:, None])
    if trace:
        install_ntff_hook()
    res = run_bass_kernel_spmd(nc, in_maps, list(range(NC_)), trace=trace)
    out = np.concatenate([r["y"] for r in res.results], axis=0)
    return out.reshape(B, L, E).astype(np.float32), res


# revision 15
# speedup vs baseline: 1.3845x; 1.3845x over previous
"""Trainium2 Bass kernel for nn_DecoderBlock (B=2, L=2048, E=1024, H=16, D=64, DFF=4096).

Strategy (8 NeuronCores, SPMD):
  - Attention is head-parallel: core c computes heads {2c, 2c+1} for all 4096
    tokens (uniform program across cores), including the out-proj partial
    contribution of its 128 hd-rows of Wo -- exchanged with a single AllToAll
    so each core ends up with the full attention output for its own 512-token
    slice.
  - Everything else (norms, residuals, GLU FFN) is token-parallel on the
    core's own 512 tokens.
  - Residual stream is kept transposed ([E on partitions, tokens free]) so
    all matmuls use natural weight layouts and biases are per-partition.
  - ALiBi is folded into the QK^T matmul as a rank-2 extension of the
    contraction dim (rows 64/65 of Qhat/Khat); causal masking multiplies
    the two diagonal-straddling k-tiles by constant 0/1 masks after exp;
    fully-masked tiles are skipped. Softmax needs no max-subtraction
    (exponents bounded by ~|S|/8 <= ~3) and normalization divides the
    V-weighted sums per head.
  - All matmuls run in float32r (full PE speed at >=256 moving free dim).
"""

import sys
import types

import numpy as np

sys.path.insert(0, "/opt/trn_rl_repo")
sys.path.insert(0, "/opt/pypackages")

import concourse.bass as bass
import concourse.mybir as mybir
from concourse import bacc
import concourse.tile as tile
from concourse.bass_utils import run_bass_kernel_spmd

F32 = mybir.dt.float32
F32R = mybir.dt.float32r
AF = mybir.ActivationFunctionType
ALU = mybir.AluOpType

B, L, E, H, D, DFF = 2, 2048, 1024, 16, 64, 4096
T = B * L            # 4096 flat tokens
NC_ = 8              # cores
TOK = T // NC_       # 512 own tokens per core
ET = E // 128        # 8 e-tiles
DT = DFF // 128      # 32 dff-tiles
QC = 256             # q-chunk width in attention
NQC = L // QC        # 8 q-chunks per sequence

_CACHE = {}


def install_ntff_hook():
    """Synthesize antenv.axon_hooks so trace=True can profile via libaxon_pjrt."""
    try:
        from antenv.axon_hooks import get_axon_ntff_profile_hook  # noqa
        return
    except ImportError:
        pass
    try:
        import antenv
        mod = types.ModuleType("antenv.axon_hooks")
        mod._hook = None
        mod.set_axon_ntff_profile_hook = lambda h: setattr(mod, "_hook", h)
        mod.get_axon_ntff_profile_hook = lambda: mod._hook
        sys.modules["antenv.axon_hooks"] = mod
        antenv.axon_hooks = mod
        if "/root/.axon_site" not in sys.path:
            sys.path.insert(0, "/root/.axon_site")
        from trn_agent_boot.trn_boot import _ntff_profile_via_ctypes
        mod.set_axon_ntff_profile_hook(
            _ntff_profile_via_ctypes("/opt/axon/libaxon_pjrt.so")
        )
    except Exception:
        pass


def build_nc():
    nc = bacc.Bacc("TRN2", target_bir_lowering=False, debug=False)

    # ---- I/O ----
    xt_full = nc.dram_tensor("xt_full", [E, T], F32R, kind="ExternalInput")
    xt_own = nc.dram_tensor("xt_own", [E, TOK], F32R, kind="ExternalInput")
    wq = nc.dram_tensor("wq", [E, 128], F32R, kind="ExternalInput")
    wk = nc.dram_tensor("wk", [E, 128], F32R, kind="ExternalInput")
    wv = nc.dram_tensor("wv", [E, 128], F32R, kind="ExternalInput")
    bqkv = nc.dram_tensor("bqkv", [128, 3], F32, kind="ExternalInput")
    wo = nc.dram_tensor("wo", [E, E], F32R, kind="ExternalInput")
    ww = nc.dram_tensor("ww", [E, DFF], F32R, kind="ExternalInput")
    wvf = nc.dram_tensor("wvf", [E, DFF], F32R, kind="ExternalInput")
    wout = nc.dram_tensor("wout", [DFF, E], F32R, kind="ExternalInput")
    posd = nc.dram_tensor("posd", [1, T], F32R, kind="ExternalInput")
    slopesd = nc.dram_tensor("slopesd", [2, 1], F32, kind="ExternalInput")
    onesd = nc.dram_tensor("onesd", [128, 1], F32R, kind="ExternalInput")
    extkd = nc.dram_tensor("extkd", [2, T], F32R, kind="ExternalInput")
    extqd = nc.dram_tensor("extqd", [2, T], F32R, kind="ExternalInput")
    extsckd = nc.dram_tensor("extsckd", [2, 2], F32, kind="ExternalInput")
    extscqd = nc.dram_tensor("extscqd", [2, 2], F32, kind="ExternalInput")
    mask0d = nc.dram_tensor("mask0d", [128, QC], F32R, kind="ExternalInput")
    mask1d = nc.dram_tensor("mask1d", [128, QC], F32R, kind="ExternalInput")
    identd = nc.dram_tensor("identd", [128, 128], F32R, kind="ExternalInput")

    y = nc.dram_tensor("y", [TOK, E], F32, kind="ExternalOutput")

    a2a_in = nc.dram_tensor("a2a_in", [NC_, TOK, 128], F32R)
    a2a_out = nc.dram_tensor("a2a_out", [NC_, TOK, 128], F32R)

    with tile.TileContext(nc) as tc:
        # ---------- constants (whole kernel) ----------
        from contextlib import ExitStack
        estack = ExitStack()
        const = estack.enter_context(tc.tile_pool(name="const", bufs=1))
        ones_sb = const.tile([128, 1], F32R)
        nc.sync.dma_start(out=ones_sb, in_=onesd[:])
        mask_sb = [const.tile([128, QC], F32R, name=f"mask{i}", tag=f"mask{i}") for i in range(2)]
        nc.sync.dma_start(out=mask_sb[0], in_=mask0d[:])
        nc.sync.dma_start(out=mask_sb[1], in_=mask1d[:])
        ident_sb = const.tile([128, 128], F32R)
        nc.sync.dma_start(out=ident_sb, in_=identd[:])
        bqkv_sb = const.tile([128, 3], F32)
        nc.sync.dma_start(out=bqkv_sb, in_=bqkv[:])

        # ---------- persistent attention state ----------
        att_stack = ExitStack()
        att_pool = att_stack.enter_context(tc.tile_pool(name="att", bufs=1))
        qhat = [att_pool.tile([66, T], F32R, name=f"qhat{h}", tag=f"qhat{h}") for h in range(2)]
        khat = [att_pool.tile([66, T], F32R, name=f"khat{h}", tag=f"khat{h}") for h in range(2)]
        vnat = att_pool.tile([128, T], F32R, tag="vnat")  # cols jt*128+d2

        # alibi extension rows (written as [2, T] blocks at partition base 64)
        with tc.tile_pool(name="extp", bufs=1) as extp:
            extk_sb = extp.tile([2, T], F32R)
            nc.sync.dma_start(out=extk_sb, in_=extkd[:])
            extq_sb = extp.tile([2, T], F32R)
            nc.sync.dma_start(out=extq_sb, in_=extqd[:])
            extsck_sb = extp.tile([2, 2], F32)
            nc.sync.dma_start(out=extsck_sb, in_=extsckd[:])
            extscq_sb = extp.tile([2, 2], F32)
            nc.sync.dma_start(out=extscq_sb, in_=extscqd[:])
            for hh in range(2):
                nc.vector.tensor_scalar(
                    out=qhat[hh][64:66, :], in0=extq_sb[:],
                    scalar1=extscq_sb[:, hh:hh + 1], scalar2=None, op0=ALU.mult,
                )
                nc.vector.tensor_scalar(
                    out=khat[hh][64:66, :], in0=extk_sb[:],
                    scalar1=extsck_sb[:, hh:hh + 1], scalar2=None, op0=ALU.mult,
                )

        # ================= Phase 1: rmsnorm1 + QKV projections =================
        with (
            tc.tile_pool(name="p1", bufs=2) as p1,
            tc.tile_pool(name="p1s", bufs=1) as p1s,
            tc.tile_pool(name="p1ps", bufs=2, space="PSUM") as p1ps,
        ):
            wq_sb = p1s.tile([128, ET, 128], F32R, tag="wq")
            wk_sb = p1s.tile([128, ET, 128], F32R, tag="wk")
            wv_sb = p1s.tile([128, ET, 128], F32R, tag="wv")
            for e in range(ET):
                nc.sync.dma_start(out=wq_sb[:, e, :], in_=wq[128 * e:128 * e + 128, :])
                nc.sync.dma_start(out=wk_sb[:, e, :], in_=wk[128 * e:128 * e + 128, :])
                nc.sync.dma_start(out=wv_sb[:, e, :], in_=wv[128 * e:128 * e + 128, :])

            for tci in range(T // 512):
                c0 = 512 * tci
                xtc = p1.tile([128, ET, 512], F32R, tag="xtc", bufs=3)
                for e in range(ET):
                    nc.sync.dma_start(
                        out=xtc[:, e, :], in_=xt_full[128 * e:128 * e + 128, c0:c0 + 512]
                    )
                ss_ps = p1ps.tile([1, 512], F32, tag="ss")
                for e in range(ET):
                    sq_e = p1.tile([128, 512], F32R, tag="sq", name="sq_e")
                    nc.scalar.activation(sq_e[:], xtc[:, e, :], AF.Square)
                    nc.tensor.matmul(
                        ss_ps[:], ones_sb[:], sq_e[:],
                        start=(e == 0), stop=(e == ET - 1),
                    )
                rinv = p1.tile([1, 512], F32R, tag="rinv")
                with nc.allow_low_precision(reason="f32r same bits as f32"):
                    nc.vector.reciprocal(rinv[:], ss_ps[:])
                r = p1.tile([1, 512], F32R, tag="r")
                nc.scalar.activation(r[:], rinv[:], AF.Sqrt, scale=float(E))
                rb = p1.tile([128, 512], F32R, tag="rb")
                nc.gpsimd.partition_broadcast(rb[:], r[:])
                h_t = p1.tile([128, ET, 512], F32R, tag="h")
                for e in range(ET):
                    nc.vector.tensor_mul(h_t[:, e, :], xtc[:, e, :], rb[:])

                # Q
                q_ps = p1ps.tile([128, 512], F32, tag="qkv")
                for e in range(ET):
                    nc.tensor.matmul(
                        q_ps[:], wq_sb[:, e, :], h_t[:, e, :],
                        start=(e == 0), stop=(e == ET - 1),
                    )
                for hh in range(2):
                    nc.scalar.activation(
                        qhat[hh][0:64, c0:c0 + 512], q_ps[64 * hh:64 * hh + 64, :],
                        AF.Identity, bias=bqkv_sb[64 * hh:64 * hh + 64, 0:1], scale=0.125,
                    )
                # K
                k_ps = p1ps.tile([128, 512], F32, tag="qkv")
                for e in range(ET):
                    nc.tensor.matmul(
                        k_ps[:], wk_sb[:, e, :], h_t[:, e, :],
                        start=(e == 0), stop=(e == ET - 1),
                    )
                for hh in range(2):
                    nc.scalar.activation(
                        khat[hh][0:64, c0:c0 + 512], k_ps[64 * hh:64 * hh + 64, :],
                        AF.Identity, bias=bqkv_sb[64 * hh:64 * hh + 64, 1:2],
                    )
                # V -> natural layout via transpose
                v_ps = p1ps.tile([128, 512], F32, tag="qkv")
                for e in range(ET):
                    nc.tensor.matmul(
                        v_ps[:], wv_sb[:, e, :], h_t[:, e, :],
                        start=(e == 0), stop=(e == ET - 1),
                    )
                vt_sb = p1.tile([128, 512], F32R, tag="vt")
                nc.scalar.activation(vt_sb[:], v_ps[:], AF.Identity, bias=bqkv_sb[:, 2:3])
                for j in range(4):
                    vtr_ps = p1ps.tile([128, 128], F32R, tag="vtr")
                    nc.tensor.transpose(
                        vtr_ps[:], vt_sb[:, 128 * j:128 * j + 128], ident_sb[:]
                    )
                    jt = 4 * tci + j
                    nc.vector.tensor_copy(vnat[:, 128 * jt:128 * jt + 128], vtr_ps[:])

        # ================= Phase 2: attention =================
        with (
            tc.tile_pool(name="p2", bufs=4) as p2,
            tc.tile_pool(name="p2b", bufs=2) as p2b,
            tc.tile_pool(name="p2ps", bufs=2, space="PSUM") as p2ps,
            tc.tile_pool(name="p2ps1", bufs=1, space="PSUM") as p2ps1,
        ):
            for hh in range(2):
                for s in range(B):
                    for qc in range(NQC):
                        q0 = s * L + QC * qc
                        nkt = 2 * (qc + 1)
                        ctx_ps = p2ps.tile([64, QC], F32, tag="ctx")
                        rs_ps = p2ps.tile([1, QC], F32, tag="rs")
                        for kt in range(nkt):
                            koff = s * L + 128 * kt
                            s_ps = p2ps.tile([128, QC], F32, tag="sps")
                            nc.tensor.matmul(
                                s_ps[:], khat[hh][:, koff:koff + 128],
                                qhat[hh][:, q0:q0 + QC], start=True, stop=True,
                            )
                            a_sb = p2.tile([128, QC], F32R, tag="a")
                            diag = kt >= 2 * qc
                            if diag:
                                # clamp so exp of to-be-masked entries can't overflow
                                s_cl = p2.tile([128, QC], F32R, tag="scl", name="s_cl")
                                nc.vector.tensor_scalar(
                                    out=s_cl[:], in0=s_ps[:], scalar1=80.0,
                                    scalar2=None, op0=ALU.min,
                                )
                                nc.scalar.activation(a_sb[:], s_cl[:], AF.Exp)
                                nc.vector.tensor_mul(
                                    a_sb[:], a_sb[:], mask_sb[kt - 2 * qc][:]
                                )
                            else:
                                nc.scalar.activation(a_sb[:], s_ps[:], AF.Exp)
                            nc.tensor.matmul(
                                rs_ps[:], ones_sb[:], a_sb[:],
                                start=(kt == 0), stop=(kt == nkt - 1),
                            )
                            jt = s * (L // 128) + kt
                            nc.tensor.matmul(
                                ctx_ps[:],
                                vnat[:, 128 * jt + 64 * hh:128 * jt + 64 * hh + 64],
                                a_sb[:], start=(kt == 0), stop=(kt == nkt - 1),
                            )
                        rinv_a = p2b.tile([1, QC], F32R, tag="rinva")
                        with nc.allow_low_precision(reason="f32r same bits"):
                            nc.vector.reciprocal(rinv_a[:], rs_ps[:])
                        bca = p2b.tile([64, QC], F32R, tag="bca")
                        nc.gpsimd.partition_broadcast(bca[:], rinv_a[:])
                        ctxn = p2b.tile([64, QC], F32R, tag="ctxn")
                        nc.vector.tensor_mul(ctxn[:], ctx_ps[:], bca[:])
                        for half in range(2):
                            tr_ps = p2ps1.tile([128, 64], F32R, tag="tr")
                            nc.tensor.transpose(
                                tr_ps[:], ctxn[:, 128 * half:128 * half + 128],
                                ident_sb[0:64, 0:64],
                            )
                            ctx_nat = p2b.tile([128, 64], F32R, tag="cnat")
                            nc.vector.tensor_copy(ctx_nat[:], tr_ps[:])
                            tglob = s * L + QC * qc + 128 * half
                            j = tglob // TOK
                            tloc = tglob % TOK
                            nc.sync.dma_start(
                                out=a2a_in[j, tloc:tloc + 128, 64 * hh:64 * hh + 64],
                                in_=ctx_nat[:],
                            )

        # ================= AllToAll =================
        nc.gpsimd.collective_compute(
            "AllToAll", ALU.bypass,
            replica_groups=[list(range(NC_))],
            ins=[a2a_in[:]], outs=[a2a_out[:]],
        )

        att_stack.close()

        # ================= Phase 3a: out-proj + residual + cross stage ========
        x_stack = ExitStack()
        x_pool = x_stack.enter_context(tc.tile_pool(name="xp", bufs=1))
        x3 = x_pool.tile([128, ET, TOK], F32R, tag="x3")
        h3 = x_pool.tile([128, ET, TOK], F32R, tag="h3")

        with (
            tc.tile_pool(name="p3", bufs=3) as p3,
            tc.tile_pool(name="p3s", bufs=1) as p3s,
            tc.tile_pool(name="p3w", bufs=4) as p3w,
            tc.tile_pool(name="p3ps", bufs=2, space="PSUM") as p3ps,
        ):
            # received ctx -> transposed [hd, tok]
            cxt = p3s.tile([128, ET, TOK], F32R, tag="cxt")
            for p in range(NC_):
                for tt in range(TOK // 128):
                    rb_sb = p3.tile([128, 128], F32R, tag="rbsb")
                    nc.sync.dma_start(
                        out=rb_sb[:], in_=a2a_out[p, 128 * tt:128 * tt + 128, :]
                    )
                    tr_ps = p3ps.tile([128, 128], F32R, tag="tr")
                    nc.tensor.transpose(tr_ps[:], rb_sb[:], ident_sb[:])
                    nc.vector.tensor_copy(
                        cxt[:, p, 128 * tt:128 * tt + 128], tr_ps[:]
                    )
            # own residual stream
            xo_sb = p3s.tile([128, ET, TOK], F32R, tag="xo")
            for e in range(ET):
                nc.sync.dma_start(
                    out=xo_sb[:, e, :], in_=xt_own[128 * e:128 * e + 128, :]
                )
            # out-proj + residual -> x2 (stored into x3 buffer temporarily? keep x2)
            x2 = p3s.tile([128, ET, TOK], F32R, tag="x2")
            for e in range(ET):
                op_ps = p3ps.tile([128, TOK], F32, tag="op")
                for p in range(ET):
                    wo_t = p3w.tile([128, 128], F32R, tag="wot")
                    nc.sync.dma_start(
                        out=wo_t[:],
                        in_=wo[128 * p:128 * p + 128, 128 * e:128 * e + 128],
                    )
                    nc.tensor.matmul(
                        op_ps[:], wo_t[:], cxt[:, p, :],
                        start=(p == 0), stop=(p == ET - 1),
                    )
                nc.vector.tensor_add(x2[:, e, :], op_ps[:], xo_sb[:, e, :])

            # cross-attn stage: x3 = x2 * (1 + rms_scale)
            ss2 = p3ps.tile([1, TOK], F32, tag="ss2")
            for e in range(ET):
                sq2 = p3.tile([128, TOK], F32R, tag="sq2")
                nc.scalar.activation(sq2[:], x2[:, e, :], AF.Square)
                nc.tensor.matmul(
                    ss2[:], ones_sb[:], sq2[:], start=(e == 0), stop=(e == ET - 1)
                )
            rinv2 = p3.tile([1, TOK], F32R, tag="rinv2")
            with nc.allow_low_precision(reason="f32r same bits"):
                nc.vector.reciprocal(rinv2[:], ss2[:])
            r2 = p3.tile([1, TOK], F32R, tag="r2")
            nc.scalar.activation(r2[:], rinv2[:], AF.Sqrt, scale=float(E))
            r2p = p3.tile([1, TOK], F32R, tag="r2p")
            nc.vector.tensor_scalar(
                out=r2p[:], in0=r2[:], scalar1=1.0, scalar2=None, op0=ALU.add
            )
            rb2 = p3.tile([128, TOK], F32R, tag="rb2")
            nc.gpsimd.partition_broadcast(rb2[:], r2p[:])
            for e in range(ET):
                nc.vector.tensor_mul(x3[:, e, :], x2[:, e, :], rb2[:])

            # rmsnorm3 -> h3
            ss3 = p3ps.tile([1, TOK], F32, tag="ss2")
            for e in range(ET):
                sq3 = p3.tile([128, TOK], F32R, tag="sq2")
                nc.scalar.activation(sq3[:], x3[:, e, :], AF.Square)
                nc.tensor.matmul(
                    ss3[:], ones_sb[:], sq3[:], start=(e == 0), stop=(e == ET - 1)
                )
            rinv3 = p3.tile([1, TOK], F32R, tag="rinv2")
            with nc.allow_low_precision(reason="f32r same bits"):
                nc.vector.reciprocal(rinv3[:], ss3[:])
            r3 = p3.tile([1, TOK], F32R, tag="r2")
            nc.scalar.activation(r3[:], rinv3[:], AF.Sqrt, scale=float(E))
            rb3 = p3.tile([128, TOK], F32R, tag="rb2")
            nc.gpsimd.partition_broadcast(rb3[:], r3[:])
            for e in range(ET):
                nc.vector.tensor_mul(h3[:, e, :], x3[:, e, :], rb3[:])

        # ================= Phase 3b: GLU FFN =================
        with (
            tc.tile_pool(name="g", bufs=1) as gpool,
            tc.tile_pool(name="f3", bufs=2) as f3,
            tc.tile_pool(name="f3w", bufs=2) as f3w,
        ):
            g = gpool.tile([128, DT, TOK], F32R, tag="g")
            NBLK = 2  # n-tiles per weight chunk
            uv_ps_pool = ExitStack()
            f3ps = uv_ps_pool.enter_context(
                tc.tile_pool(name="f3ps", bufs=3, space="PSUM")
            )
            for nb in range(DT // NBLK):
                ww_c = f3w.tile([128, ET, NBLK * 128], F32R, tag="wwc", bufs=3)
                wv_c = f3w.tile([128, ET, NBLK * 128], F32R, tag="wvc", bufs=3)
                for e in range(ET):
                    nc.sync.dma_start(
                        out=ww_c[:, e, :],
                        in_=ww[128 * e:128 * e + 128,
                               NBLK * 128 * nb:NBLK * 128 * (nb + 1)],
                    )
                    nc.sync.dma_start(
                        out=wv_c[:, e, :],
                        in_=wvf[128 * e:128 * e + 128,
                                NBLK * 128 * nb:NBLK * 128 * (nb + 1)],
                    )
                for j in range(NBLK):
                    n = NBLK * nb + j
                    u_ps = f3ps.tile([128, TOK], F32, tag="mm")
                    for e in range(ET):
                        nc.tensor.matmul(
                            u_ps[:], ww_c[:, e, 128 * j:128 * j + 128], h3[:, e, :],
                            start=(e == 0), stop=(e == ET - 1),
                        )
                    nc.scalar.activation(g[:, n, :], u_ps[:], AF.Gelu)
                    v_ps = f3ps.tile([128, TOK], F32, tag="mm")
                    for e in range(ET):
                        nc.tensor.matmul(
                            v_ps[:], wv_c[:, e, 128 * j:128 * j + 128], h3[:, e, :],
                            start=(e == 0), stop=(e == ET - 1),
                        )
                    vtmp = f3.tile([128, TOK], F32R, tag="vtmp")
                    nc.vector.tensor_copy(vtmp[:], v_ps[:])
                    nc.vector.tensor_mul(g[:, n, :], g[:, n, :], vtmp[:])

            # Wout + residual -> x4: d-outer with 8 PSUM accumulators and
            # contiguous [128, E] weight-row DMAs (prefetched)
            x4 = gpool.tile([128, ET, TOK], F32R, tag="x4")
            uv_ps_pool.close()
            with tc.tile_pool(name="fps8", bufs=1, space="PSUM") as fps8:
                f_ps = [
                    fps8.tile([128, TOK], F32, name=f"fps{e}", tag=f"fps{e}")
                    for e in range(ET)
                ]
                for d in range(DT):
                    wt = f3w.tile([128, E], F32R, tag="wot2", bufs=3, name="wt")
                    nc.sync.dma_start(out=wt[:], in_=wout[128 * d:128 * d + 128, :])
                    for e in range(ET):
                        nc.tensor.matmul(
                            f_ps[e][:], wt[:, 128 * e:128 * e + 128], g[:, d, :],
                            start=(d == 0), stop=(d == DT - 1),
                        )
                for e in range(ET):
                    nc.vector.tensor_add(x4[:, e, :], f_ps[e][:], x3[:, e, :])

            # output transpose + DMA
            with tc.tile_pool(name="otrps", bufs=2, space="PSUM") as otrpool:
                for tt in range(TOK // 128):
                    out_nat = f3.tile([128, E], F32, tag="onat")
                    for e in range(ET):
                        tr_ps = otrpool.tile([128, 128], F32R, tag="otr")
                        nc.tensor.transpose(
                            tr_ps[:], x4[:, e, 128 * tt:128 * tt + 128], ident_sb[:]
                        )
                        nc.vector.tensor_copy(
                            out_nat[:, 128 * e:128 * e + 128], tr_ps[:]
                        )
                    nc.sync.dma_start(
                        out=y[128 * tt:128 * tt + 128, :], in_=out_nat[:]
                    )

        x_stack.close()
        estack.close()

    nc.finalize()
    return nc


def make_in_maps(X, Wqkv, bqkv, Wo_sa, Ww, Wv, Wout):
    Xf = np.ascontiguousarray(X.reshape(T, E).astype(np.float32))
    XT = np.ascontiguousarray(Xf.T)  # [E, T]
    Wr = Wqkv.reshape(E, H, 3, D)
    br = bqkv.reshape(H, 3, D)
    pos = (np.arange(T, dtype=np.float32) % L)[None, :]
    slopes_all = (2.0 ** (-np.linspace(1.0, 8.0, H))).astype(np.float32)
    ones_col = np.ones([128, 1], np.float32)
    onesrow = np.ones([1, T], np.float32)
    extk = np.concatenate([pos, onesrow], axis=0).astype(np.float32)   # [2, T]
    extq = np.concatenate([onesrow, pos], axis=0).astype(np.float32)
    p_i = np.arange(128)[:, None]
    f_i = np.arange(QC)[None, :]
    mask0 = (f_i - p_i >= 0).astype(np.float32)
    mask1 = (f_i - p_i - 128 >= 0).astype(np.float32)
    ident = np.eye(128, dtype=np.float32)

    in_maps = []
    for c in range(NC_):
        h0 = 2 * c
        wq_p = np.ascontiguousarray(
            np.concatenate([Wr[:, h0, 0, :], Wr[:, h0 + 1, 0, :]], axis=1)
        )
        wk_p = np.ascontiguousarray(
            np.concatenate([Wr[:, h0, 1, :], Wr[:, h0 + 1, 1, :]], axis=1)
        )
        wv_p = np.ascontiguousarray(
            np.concatenate([Wr[:, h0, 2, :], Wr[:, h0 + 1, 2, :]], axis=1)
        )
        bq_p = np.concatenate([br[h0, 0], br[h0 + 1, 0]]) * 0.125
        bk_p = np.concatenate([br[h0, 1], br[h0 + 1, 1]])
        bv_p = np.concatenate([br[h0, 2], br[h0 + 1, 2]])
        bq3 = np.stack([bq_p, bk_p, bv_p], axis=1).astype(np.float32)  # [128,3]
        xt_own = np.ascontiguousarray(XT[:, TOK * c:TOK * (c + 1)])
        in_maps.append({
            "xt_full": XT,
            "xt_own": xt_own,
            "wq": wq_p.astype(np.float32),
            "wk": wk_p.astype(np.float32),
            "wv": wv_p.astype(np.float32),
            "bqkv": bq3,
            "wo": np.ascontiguousarray(Wo_sa.astype(np.float32)),
            "ww": np.ascontiguousarray(Ww.astype(np.float32)),
            "wvf": np.ascontiguousarray(Wv.astype(np.float32)),
            "wout": np.ascontiguousarray(Wout.astype(np.float32)),
            "posd": pos,
            "slopesd": slopes_all[h0:h0 + 2].reshape(2, 1),
            "onesd": ones_col,
            "extkd": extk,
            "extqd": extq,
            "extsckd": np.stack(
                [[slopes_all[h0], 1.0], [slopes_all[h0 + 1], 1.0]], axis=1
            ).astype(np.float32),
            "extscqd": np.stack(
                [[1.0, -slopes_all[h0]], [1.0, -slopes_all[h0 + 1]]], axis=1
            ).astype(np.float32),
            "mask0d": mask0,
            "mask1d": mask1,
            "identd": ident,
        })
    return in_maps


def kernel(**inputs) -> np.ndarray:
    out, _ = run(inputs, trace=False)
    return out


def run(inputs, trace=False):
    """bo_sa is folded into xt_own host-side (it is a constant per-channel
    add to the residual stream right where xt_own enters)."""
    if "nc" not in _CACHE:
        _CACHE["nc"] = build_nc()
    nc = _CACHE["nc"]
    in_maps = make_in_maps(
        inputs["X"], inputs["Wqkv"], inputs["bqkv"], inputs["Wo_sa"],
        inputs["Ww"], inputs["Wv"], inputs["Wout"],
    )
    bo = inputs["bo_sa"].astype(np.float32)  # [E]
    for m in in_maps:
        m["xt_own"] = np.ascontiguousarray(m["xt_own"] + bo[:, None])
    if trace:
        install_ntff_hook()
    res = run_bass_kernel_spmd(nc, in_maps, list(range(NC_)), trace=trace)
    out = np.concatenate([r["y"] for r in res.results], axis=0)
    return out.reshape(B, L, E).astype(np.float32), res


# revision 16
# speedup vs baseline: 1.4997x; 1.0832x over previous
"""Trainium2 Bass kernel for nn_DecoderBlock (B=2, L=2048, E=1024, H=16, D=64, DFF=4096).

Strategy (8 NeuronCores, SPMD):
  - Attention is head-parallel: core c computes heads {2c, 2c+1} for all 4096
    tokens (uniform program across cores), including the out-proj partial
    contribution of its 128 hd-rows of Wo -- exchanged with a single AllToAll
    so each core ends up with the full attention output for its own 512-token
    slice.
  - Everything else (norms, residuals, GLU FFN) is token-parallel on the
    core's own 512 tokens.
  - Residual stream is kept transposed ([E on partitions, tokens free]) so
    all matmuls use natural weight layouts and biases are per-partition.
  - ALiBi is folded into the QK^T matmul as a rank-2 extension of the
    contraction dim (rows 64/65 of Qhat/Khat); causal masking multiplies
    the two diagonal-straddling k-tiles by constant 0/1 masks after exp;
    fully-masked tiles are skipped. Softmax needs no max-subtraction
    (exponents bounded by ~|S|/8 <= ~3) and normalization divides the
    V-weighted sums per head.
  - All matmuls run in float32r (full PE speed at >=256 moving free dim).
"""

import sys
import types

import numpy as np

sys.path.insert(0, "/opt/trn_rl_repo")
sys.path.insert(0, "/opt/pypackages")

import concourse.bass as bass
import concourse.mybir as mybir
from concourse import bacc
import concourse.tile as tile
from concourse.bass_utils import run_bass_kernel_spmd

F32 = mybir.dt.float32
F32R = mybir.dt.float32r
AF = mybir.ActivationFunctionType
ALU = mybir.AluOpType

B, L, E, H, D, DFF = 2, 2048, 1024, 16, 64, 4096
T = B * L            # 4096 flat tokens
NC_ = 8              # cores
TOK = T // NC_       # 512 own tokens per core
ET = E // 128        # 8 e-tiles
DT = DFF // 128      # 32 dff-tiles
QC = 256             # q-chunk width in attention
NQC = L // QC        # 8 q-chunks per sequence

_CACHE = {}


def install_ntff_hook():
    """Synthesize antenv.axon_hooks so trace=True can profile via libaxon_pjrt."""
    try:
        from antenv.axon_hooks import get_axon_ntff_profile_hook  # noqa
        return
    except ImportError:
        pass
    try:
        import antenv
        mod = types.ModuleType("antenv.axon_hooks")
        mod._hook = None
        mod.set_axon_ntff_profile_hook = lambda h: setattr(mod, "_hook", h)
        mod.get_axon_ntff_profile_hook = lambda: mod._hook
        sys.modules["antenv.axon_hooks"] = mod
        antenv.axon_hooks = mod
        if "/root/.axon_site" not in sys.path:
            sys.path.insert(0, "/root/.axon_site")
        from trn_agent_boot.trn_boot import _ntff_profile_via_ctypes
        mod.set_axon_ntff_profile_hook(
            _ntff_profile_via_ctypes("/opt/axon/libaxon_pjrt.so")
        )
    except Exception:
        pass


def build_nc():
    nc = bacc.Bacc("TRN2", target_bir_lowering=False, debug=False)

    # ---- I/O ----
    xt_full = nc.dram_tensor("xt_full", [E, T], F32R, kind="ExternalInput")
    xt_own = nc.dram_tensor("xt_own", [E, TOK], F32R, kind="ExternalInput")
    wq = nc.dram_tensor("wq", [E, 128], F32R, kind="ExternalInput")
    wk = nc.dram_tensor("wk", [E, 128], F32R, kind="ExternalInput")
    wv = nc.dram_tensor("wv", [E, 128], F32R, kind="ExternalInput")
    bqkv = nc.dram_tensor("bqkv", [128, 3], F32, kind="ExternalInput")
    wo = nc.dram_tensor("wo", [E, E], F32R, kind="ExternalInput")
    ww = nc.dram_tensor("ww", [E, DFF], F32R, kind="ExternalInput")
    wvf = nc.dram_tensor("wvf", [E, DFF], F32R, kind="ExternalInput")
    wout = nc.dram_tensor("wout", [DFF, E], F32R, kind="ExternalInput")
    posd = nc.dram_tensor("posd", [1, T], F32R, kind="ExternalInput")
    slopesd = nc.dram_tensor("slopesd", [2, 1], F32, kind="ExternalInput")
    onesd = nc.dram_tensor("onesd", [128, 1], F32R, kind="ExternalInput")
    extkd = nc.dram_tensor("extkd", [2, T], F32R, kind="ExternalInput")
    extqd = nc.dram_tensor("extqd", [2, T], F32R, kind="ExternalInput")
    extsckd = nc.dram_tensor("extsckd", [2, 2], F32, kind="ExternalInput")
    extscqd = nc.dram_tensor("extscqd", [2, 2], F32, kind="ExternalInput")
    mask0d = nc.dram_tensor("mask0d", [128, QC], F32R, kind="ExternalInput")
    mask1d = nc.dram_tensor("mask1d", [128, QC], F32R, kind="ExternalInput")
    identd = nc.dram_tensor("identd", [128, 128], F32R, kind="ExternalInput")
    vonesd = nc.dram_tensor("vonesd", [128, 64], F32R, kind="ExternalInput")

    y = nc.dram_tensor("y", [TOK, E], F32, kind="ExternalOutput")

    a2a_in = nc.dram_tensor("a2a_in", [NC_, 128, TOK], F32R)
    a2a_out = nc.dram_tensor("a2a_out", [NC_, 128, TOK], F32R)

    with tile.TileContext(nc) as tc:
        # ---------- constants (whole kernel) ----------
        from contextlib import ExitStack
        estack = ExitStack()
        const = estack.enter_context(tc.tile_pool(name="const", bufs=1))
        ones_sb = const.tile([128, 1], F32R)
        nc.sync.dma_start(out=ones_sb, in_=onesd[:])
        mask_sb = [const.tile([128, QC], F32R, name=f"mask{i}", tag=f"mask{i}") for i in range(2)]
        nc.sync.dma_start(out=mask_sb[0], in_=mask0d[:])
        nc.sync.dma_start(out=mask_sb[1], in_=mask1d[:])
        ident_sb = const.tile([128, 128], F32R)
        nc.sync.dma_start(out=ident_sb, in_=identd[:])
        bqkv_sb = const.tile([128, 3], F32)
        nc.sync.dma_start(out=bqkv_sb, in_=bqkv[:])

        # ---------- persistent attention state ----------
        att_stack = ExitStack()
        att_pool = att_stack.enter_context(tc.tile_pool(name="att", bufs=1))
        qhat = [att_pool.tile([66, T], F32R, name=f"qhat{h}", tag=f"qhat{h}") for h in range(2)]
        khat = [att_pool.tile([66, T], F32R, name=f"khat{h}", tag=f"khat{h}") for h in range(2)]
        vnat = att_pool.tile([128, T // 128, 2, 65], F32R, tag="vnat")
        nc.sync.dma_start(out=vnat[:, :, :, 64:65], in_=vonesd[:])

        # alibi extension rows (written as [2, T] blocks at partition base 64)
        with tc.tile_pool(name="extp", bufs=1) as extp:
            extk_sb = extp.tile([2, T], F32R)
            nc.sync.dma_start(out=extk_sb, in_=extkd[:])
            extq_sb = extp.tile([2, T], F32R)
            nc.sync.dma_start(out=extq_sb, in_=extqd[:])
            extsck_sb = extp.tile([2, 2], F32)
            nc.sync.dma_start(out=extsck_sb, in_=extsckd[:])
            extscq_sb = extp.tile([2, 2], F32)
            nc.sync.dma_start(out=extscq_sb, in_=extscqd[:])
            for hh in range(2):
                nc.vector.tensor_scalar(
                    out=qhat[hh][64:66, :], in0=extq_sb[:],
                    scalar1=extscq_sb[:, hh:hh + 1], scalar2=None, op0=ALU.mult,
                )
                nc.vector.tensor_scalar(
                    out=khat[hh][64:66, :], in0=extk_sb[:],
                    scalar1=extsck_sb[:, hh:hh + 1], scalar2=None, op0=ALU.mult,
                )

        # ================= Phase 1: rmsnorm1 + QKV projections =================
        with (
            tc.tile_pool(name="p1", bufs=2) as p1,
            tc.tile_pool(name="p1s", bufs=1) as p1s,
            tc.tile_pool(name="p1ps", bufs=2, space="PSUM") as p1ps,
        ):
            wq_sb = p1s.tile([128, ET, 128], F32R, tag="wq")
            wk_sb = p1s.tile([128, ET, 128], F32R, tag="wk")
            wv_sb = p1s.tile([128, ET, 128], F32R, tag="wv")
            for e in range(ET):
                nc.sync.dma_start(out=wq_sb[:, e, :], in_=wq[128 * e:128 * e + 128, :])
                nc.sync.dma_start(out=wk_sb[:, e, :], in_=wk[128 * e:128 * e + 128, :])
                nc.sync.dma_start(out=wv_sb[:, e, :], in_=wv[128 * e:128 * e + 128, :])

            for tci in range(T // 512):
                c0 = 512 * tci
                xtc = p1.tile([128, ET, 512], F32R, tag="xtc", bufs=3)
                for e in range(ET):
                    nc.sync.dma_start(
                        out=xtc[:, e, :], in_=xt_full[128 * e:128 * e + 128, c0:c0 + 512]
                    )
                ss_ps = p1ps.tile([1, 512], F32, tag="ss")
                for e in range(ET):
                    sq_e = p1.tile([128, 512], F32R, tag="sq", name="sq_e")
                    nc.scalar.activation(sq_e[:], xtc[:, e, :], AF.Square)
                    nc.tensor.matmul(
                        ss_ps[:], ones_sb[:], sq_e[:],
                        start=(e == 0), stop=(e == ET - 1),
                    )
                rinv = p1.tile([1, 512], F32R, tag="rinv")
                with nc.allow_low_precision(reason="f32r same bits as f32"):
                    nc.vector.reciprocal(rinv[:], ss_ps[:])
                r = p1.tile([1, 512], F32R, tag="r")
                nc.scalar.activation(r[:], rinv[:], AF.Sqrt, scale=float(E))
                rb = p1.tile([128, 512], F32R, tag="rb")
                nc.gpsimd.partition_broadcast(rb[:], r[:])
                h_t = p1.tile([128, ET, 512], F32R, tag="h")
                for e in range(ET):
                    nc.vector.tensor_mul(h_t[:, e, :], xtc[:, e, :], rb[:])

                # Q
                q_ps = p1ps.tile([128, 512], F32, tag="qkv")
                for e in range(ET):
                    nc.tensor.matmul(
                        q_ps[:], wq_sb[:, e, :], h_t[:, e, :],
                        start=(e == 0), stop=(e == ET - 1),
                    )
                for hh in range(2):
                    nc.scalar.activation(
                        qhat[hh][0:64, c0:c0 + 512], q_ps[64 * hh:64 * hh + 64, :],
                        AF.Identity, bias=bqkv_sb[64 * hh:64 * hh + 64, 0:1], scale=0.125,
                    )
                # K
                k_ps = p1ps.tile([128, 512], F32, tag="qkv")
                for e in range(ET):
                    nc.tensor.matmul(
                        k_ps[:], wk_sb[:, e, :], h_t[:, e, :],
                        start=(e == 0), stop=(e == ET - 1),
                    )
                for hh in range(2):
                    nc.scalar.activation(
                        khat[hh][0:64, c0:c0 + 512], k_ps[64 * hh:64 * hh + 64, :],
                        AF.Identity, bias=bqkv_sb[64 * hh:64 * hh + 64, 1:2],
                    )
                # V -> natural layout via transpose
                v_ps = p1ps.tile([128, 512], F32, tag="qkv")
                for e in range(ET):
                    nc.tensor.matmul(
                        v_ps[:], wv_sb[:, e, :], h_t[:, e, :],
                        start=(e == 0), stop=(e == ET - 1),
                    )
                vt_sb = p1.tile([128, 512], F32R, tag="vt")
                nc.scalar.activation(vt_sb[:], v_ps[:], AF.Identity, bias=bqkv_sb[:, 2:3])
                for j in range(4):
                    vtr_ps = p1ps.tile([128, 128], F32R, tag="vtr")
                    nc.tensor.transpose(
                        vtr_ps[:], vt_sb[:, 128 * j:128 * j + 128], ident_sb[:]
                    )
                    jt = 4 * tci + j
                    nc.vector.tensor_copy(vnat[:, jt, 0, 0:64], vtr_ps[:, 0:64])
                    nc.vector.tensor_copy(vnat[:, jt, 1, 0:64], vtr_ps[:, 64:128])

        # ================= Phase 2: attention =================
        with (
            tc.tile_pool(name="p2", bufs=4) as p2,
            tc.tile_pool(name="p2b", bufs=2) as p2b,
            tc.tile_pool(name="p2ps", bufs=2, space="PSUM") as p2ps,
            tc.tile_pool(name="p2ps1", bufs=1, space="PSUM") as p2ps1,
        ):
            for hh in range(2):
                for s in range(B):
                    for qc in range(NQC):
                        q0 = s * L + QC * qc
                        nkt = 2 * (qc + 1)
                        ctx_ps = p2ps.tile([65, QC], F32, tag="ctx")
                        for kt in range(nkt):
                            koff = s * L + 128 * kt
                            s_ps = p2ps.tile([128, QC], F32, tag="sps")
                            nc.tensor.matmul(
                                s_ps[:], khat[hh][:, koff:koff + 128],
                                qhat[hh][:, q0:q0 + QC], start=True, stop=True,
                            )
                            a_sb = p2.tile([128, QC], F32R, tag="a")
                            diag = kt >= 2 * qc
                            if diag:
                                # clamp so exp of to-be-masked entries can't overflow
                                s_cl = p2.tile([128, QC], F32R, tag="scl", name="s_cl")
                                nc.vector.tensor_scalar(
                                    out=s_cl[:], in0=s_ps[:], scalar1=80.0,
                                    scalar2=None, op0=ALU.min,
                                )
                                nc.scalar.activation(a_sb[:], s_cl[:], AF.Exp)
                                nc.vector.tensor_mul(
                                    a_sb[:], a_sb[:], mask_sb[kt - 2 * qc][:]
                                )
                            else:
                                nc.scalar.activation(a_sb[:], s_ps[:], AF.Exp)
                            jt = s * (L // 128) + kt
                            nc.tensor.matmul(
                                ctx_ps[:], vnat[:, jt, hh, 0:65],
                                a_sb[:], start=(kt == 0), stop=(kt == nkt - 1),
                            )
                        rinv_a = p2b.tile([1, QC], F32R, tag="rinva")
                        with nc.allow_low_precision(reason="f32r same bits"):
                            nc.vector.reciprocal(rinv_a[:], ctx_ps[64:65, :])
                        bca = p2b.tile([64, QC], F32R, tag="bca")
                        nc.gpsimd.partition_broadcast(bca[:], rinv_a[:])
                        ctxn = p2b.tile([64, QC], F32R, tag="ctxn")
                        nc.vector.tensor_mul(ctxn[:], ctx_ps[0:64, :], bca[:])
                        tglob = s * L + QC * qc
                        j = tglob // TOK
                        tloc = tglob % TOK
                        nc.sync.dma_start(
                            out=a2a_in[j, 64 * hh:64 * hh + 64, tloc:tloc + QC],
                            in_=ctxn[:],
                        )

        # ================= AllToAll =================
        nc.gpsimd.collective_compute(
            "AllToAll", ALU.bypass,
            replica_groups=[list(range(NC_))],
            ins=[a2a_in[:]], outs=[a2a_out[:]],
        )

        att_stack.close()

        # ================= Phase 3a: out-proj + residual + cross stage ========
        x_stack = ExitStack()
        x_pool = x_stack.enter_context(tc.tile_pool(name="xp", bufs=1))
        x3 = x_pool.tile([128, ET, TOK], F32R, tag="x3")
        h3 = x_pool.tile([128, ET, TOK], F32R, tag="h3")

        with (
            tc.tile_pool(name="p3", bufs=3) as p3,
            tc.tile_pool(name="p3s", bufs=1) as p3s,
            tc.tile_pool(name="p3w", bufs=4) as p3w,
            tc.tile_pool(name="p3ps", bufs=2, space="PSUM") as p3ps,
        ):
            # received ctx is already [hd, tok] per source block
            cxt = p3s.tile([128, ET, TOK], F32R, tag="cxt")
            for p in range(NC_):
                nc.sync.dma_start(out=cxt[:, p, :], in_=a2a_out[p, :, :])
            # own residual stream
            xo_sb = p3s.tile([128, ET, TOK], F32R, tag="xo")
            for e in range(ET):
                nc.sync.dma_start(
                    out=xo_sb[:, e, :], in_=xt_own[128 * e:128 * e + 128, :]
                )
            # out-proj + residual -> x2 (stored into x3 buffer temporarily? keep x2)
            x2 = p3s.tile([128, ET, TOK], F32R, tag="x2")
            for e in range(ET):
                op_ps = p3ps.tile([128, TOK], F32, tag="op")
                for p in range(ET):
                    wo_t = p3w.tile([128, 128], F32R, tag="wot")
                    nc.sync.dma_start(
                        out=wo_t[:],
                        in_=wo[128 * p:128 * p + 128, 128 * e:128 * e + 128],
                    )
                    nc.tensor.matmul(
                        op_ps[:], wo_t[:], cxt[:, p, :],
                        start=(p == 0), stop=(p == ET - 1),
                    )
                nc.vector.tensor_add(x2[:, e, :], op_ps[:], xo_sb[:, e, :])

            # cross-attn stage: x3 = x2 * (1 + rms_scale)
            ss2 = p3ps.tile([1, TOK], F32, tag="ss2")
            for e in range(ET):
                sq2 = p3.tile([128, TOK], F32R, tag="sq2")
                nc.scalar.activation(sq2[:], x2[:, e, :], AF.Square)
                nc.tensor.matmul(
                    ss2[:], ones_sb[:], sq2[:], start=(e == 0), stop=(e == ET - 1)
                )
            rinv2 = p3.tile([1, TOK], F32R, tag="rinv2")
            with nc.allow_low_precision(reason="f32r same bits"):
                nc.vector.reciprocal(rinv2[:], ss2[:])
            r2 = p3.tile([1, TOK], F32R, tag="r2")
            nc.scalar.activation(r2[:], rinv2[:], AF.Sqrt, scale=float(E))
            r2p = p3.tile([1, TOK], F32R, tag="r2p")
            nc.vector.tensor_scalar(
                out=r2p[:], in0=r2[:], scalar1=1.0, scalar2=None, op0=ALU.add
            )
            rb2 = p3.tile([128, TOK], F32R, tag="rb2")
            nc.gpsimd.partition_broadcast(rb2[:], r2p[:])
            for e in range(ET):
                nc.vector.tensor_mul(x3[:, e, :], x2[:, e, :], rb2[:])

            # rmsnorm3 -> h3
            ss3 = p3ps.tile([1, TOK], F32, tag="ss2")
            for e in range(ET):
                sq3 = p3.tile([128, TOK], F32R, tag="sq2")
                nc.scalar.activation(sq3[:], x3[:, e, :], AF.Square)
                nc.tensor.matmul(
                    ss3[:], ones_sb[:], sq3[:], start=(e == 0), stop=(e == ET - 1)
                )
            rinv3 = p3.tile([1, TOK], F32R, tag="rinv2")
            with nc.allow_low_precision(reason="f32r same bits"):
                nc.vector.reciprocal(rinv3[:], ss3[:])
            r3 = p3.tile([1, TOK], F32R, tag="r2")
            nc.scalar.activation(r3[:], rinv3[:], AF.Sqrt, scale=float(E))
            rb3 = p3.tile([128, TOK], F32R, tag="rb2")
            nc.gpsimd.partition_broadcast(rb3[:], r3[:])
            for e in range(ET):
                nc.vector.tensor_mul(h3[:, e, :], x3[:, e, :], rb3[:])

        # ================= Phase 3b: GLU FFN =================
        with (
            tc.tile_pool(name="g", bufs=1) as gpool,
            tc.tile_pool(name="f3", bufs=2) as f3,
            tc.tile_pool(name="f3w", bufs=2) as f3w,
        ):
            g = gpool.tile([128, DT, TOK], F32R, tag="g")
            NBLK = 2  # n-tiles per weight chunk
            uv_ps_pool = ExitStack()
            f3ps = uv_ps_pool.enter_context(
                tc.tile_pool(name="f3ps", bufs=3, space="PSUM")
            )
            for nb in range(DT // NBLK):
                ww_c = f3w.tile([128, ET, NBLK * 128], F32R, tag="wwc", bufs=3)
                wv_c = f3w.tile([128, ET, NBLK * 128], F32R, tag="wvc", bufs=3)
                for e in range(ET):
                    nc.sync.dma_start(
                        out=ww_c[:, e, :],
                        in_=ww[128 * e:128 * e + 128,
                               NBLK * 128 * nb:NBLK * 128 * (nb + 1)],
                    )
                    nc.sync.dma_start(
                        out=wv_c[:, e, :],
                        in_=wvf[128 * e:128 * e + 128,
                                NBLK * 128 * nb:NBLK * 128 * (nb + 1)],
                    )
                for j in range(NBLK):
                    n = NBLK * nb + j
                    u_ps = f3ps.tile([128, TOK], F32, tag="mm")
                    for e in range(ET):
                        nc.tensor.matmul(
                            u_ps[:], ww_c[:, e, 128 * j:128 * j + 128], h3[:, e, :],
                            start=(e == 0), stop=(e == ET - 1),
                        )
                    nc.scalar.activation(g[:, n, :], u_ps[:], AF.Gelu)
                    v_ps = f3ps.tile([128, TOK], F32, tag="mm")
                    for e in range(ET):
                        nc.tensor.matmul(
                            v_ps[:], wv_c[:, e, 128 * j:128 * j + 128], h3[:, e, :],
                            start=(e == 0), stop=(e == ET - 1),
                        )
                    vtmp = f3.tile([128, TOK], F32R, tag="vtmp")
                    nc.vector.tensor_copy(vtmp[:], v_ps[:])
                    nc.vector.tensor_mul(g[:, n, :], g[:, n, :], vtmp[:])

            # Wout + residual -> x4: d-outer with 8 PSUM accumulators and
            # contiguous [128, E] weight-row DMAs (prefetched)
            x4 = gpool.tile([128, ET, TOK], F32R, tag="x4")
            uv_ps_pool.close()
            with tc.tile_pool(name="fps8", bufs=1, space="PSUM") as fps8:
                f_ps = [
                    fps8.tile([128, TOK], F32, name=f"fps{e}", tag=f"fps{e}")
                    for e in range(ET)
                ]
                for d in range(DT):
                    wt = f3w.tile([128, E], F32R, tag="wot2", bufs=3, name="wt")
                    nc.sync.dma_start(out=wt[:], in_=wout[128 * d:128 * d + 128, :])
                    for e in range(ET):
                        nc.tensor.matmul(
                            f_ps[e][:], wt[:, 128 * e:128 * e + 128], g[:, d, :],
                            start=(d == 0), stop=(d == DT - 1),
                        )
                for e in range(ET):
                    nc.vector.tensor_add(x4[:, e, :], f_ps[e][:], x3[:, e, :])

            # output transpose + DMA
            with tc.tile_pool(name="otrps", bufs=2, space="PSUM") as otrpool:
                for tt in range(TOK // 128):
                    out_nat = f3.tile([128, E], F32, tag="onat")
                    for e in range(ET):
                        tr_ps = otrpool.tile([128, 128], F32R, tag="otr")
                        nc.tensor.transpose(
                            tr_ps[:], x4[:, e, 128 * tt:128 * tt + 128], ident_sb[:]
                        )
                        nc.vector.tensor_copy(
                            out_nat[:, 128 * e:128 * e + 128], tr_ps[:]
                        )
                    nc.sync.dma_start(
                        out=y[128 * tt:128 * tt + 128, :], in_=out_nat[:]
                    )

        x_stack.close()
        estack.close()

    nc.finalize()
    return nc


def make_in_maps(X, Wqkv, bqkv, Wo_sa, Ww, Wv, Wout):
    Xf = np.ascontiguousarray(X.reshape(T, E).astype(np.float32))
    XT = np.ascontiguousarray(Xf.T)  # [E, T]
    Wr = Wqkv.reshape(E, H, 3, D)
    br = bqkv.reshape(H, 3, D)
    pos = (np.arange(T, dtype=np.float32) % L)[None, :]
    slopes_all = (2.0 ** (-np.linspace(1.0, 8.0, H))).astype(np.float32)
    ones_col = np.ones([128, 1], np.float32)
    onesrow = np.ones([1, T], np.float32)
    extk = np.concatenate([pos, onesrow], axis=0).astype(np.float32)   # [2, T]
    extq = np.concatenate([onesrow, pos], axis=0).astype(np.float32)
    p_i = np.arange(128)[:, None]
    f_i = np.arange(QC)[None, :]
    mask0 = (f_i - p_i >= 0).astype(np.float32)
    mask1 = (f_i - p_i - 128 >= 0).astype(np.float32)
    ident = np.eye(128, dtype=np.float32)

    in_maps = []
    for c in range(NC_):
        h0 = 2 * c
        wq_p = np.ascontiguousarray(
            np.concatenate([Wr[:, h0, 0, :], Wr[:, h0 + 1, 0, :]], axis=1)
        )
        wk_p = np.ascontiguousarray(
            np.concatenate([Wr[:, h0, 1, :], Wr[:, h0 + 1, 1, :]], axis=1)
        )
        wv_p = np.ascontiguousarray(
            np.concatenate([Wr[:, h0, 2, :], Wr[:, h0 + 1, 2, :]], axis=1)
        )
        bq_p = np.concatenate([br[h0, 0], br[h0 + 1, 0]]) * 0.125
        bk_p = np.concatenate([br[h0, 1], br[h0 + 1, 1]])
        bv_p = np.concatenate([br[h0, 2], br[h0 + 1, 2]])
        bq3 = np.stack([bq_p, bk_p, bv_p], axis=1).astype(np.float32)  # [128,3]
        xt_own = np.ascontiguousarray(XT[:, TOK * c:TOK * (c + 1)])
        in_maps.append({
            "xt_full": XT,
            "xt_own": xt_own,
            "wq": wq_p.astype(np.float32),
            "wk": wk_p.astype(np.float32),
            "wv": wv_p.astype(np.float32),
            "bqkv": bq3,
            "wo": np.ascontiguousarray(Wo_sa.astype(np.float32)),
            "ww": np.ascontiguousarray(Ww.astype(np.float32)),
            "wvf": np.ascontiguousarray(Wv.astype(np.float32)),
            "wout": np.ascontiguousarray(Wout.astype(np.float32)),
            "posd": pos,
            "slopesd": slopes_all[h0:h0 + 2].reshape(2, 1),
            "onesd": ones_col,
            "extkd": extk,
            "extqd": extq,
            "extsckd": np.stack(
                [[slopes_all[h0], 1.0], [slopes_all[h0 + 1], 1.0]], axis=1
            ).astype(np.float32),
            "extscqd": np.stack(
                [[1.0, -slopes_all[h0]], [1.0, -slopes_all[h0 + 1]]], axis=1
            ).astype(np.float32),
            "mask0d": mask0,
            "vonesd": np.ones([128, 64], np.float32),
            "mask1d": mask1,
            "identd": ident,
        })
    return in_maps


def kernel(**inputs) -> np.ndarray:
    out, _ = run(inputs, trace=False)
    return out


def run(inputs, trace=False):
    """bo_sa is folded into xt_own host-side (it is a constant per-channel
    add to the residual stream right where xt_own enters)."""
    if "nc" not in _CACHE:
        _CACHE["nc"] = build_nc()
    nc = _CACHE["nc"]
    in_maps = make_in_maps(
        inputs["X"], inputs["Wqkv"], inputs["bqkv"], inputs["Wo_sa"],
        inputs["Ww"], inputs["Wv"], inputs["Wout"],
    )
    bo = inputs["bo_sa"].astype(np.float32)  # [E]
    for m in in_maps:
        m["xt_own"] = np.ascontiguousarray(m["xt_own"] + bo[:, None])
    if trace:
        install_ntff_hook()
    res = run_bass_kernel_spmd(nc, in_maps, list(range(NC_)), trace=trace)
    out = np.concatenate([r["y"] for r in res.results], axis=0)
    return out.reshape(B, L, E).astype(np.float32), res
